# revision 11
# baseline (speedup 1.0000x reference)
"""Trainium2 Bass kernel for a 2-layer GATv2 aggregator (N=50000, E=800000).

Self-contained: kernel(**inputs) takes full inputs, shards across 8
NeuronCores internally, returns the full (50000, 128) float32 output.
"""
"""GATv2 2-layer Trainium kernel: host prep + bass program builder + runner.

Strategy (8-core SPMD):
- dst-shard nodes across cores; edges sorted by dst, grouped into 128-edge
  chunks per (128-node block, src-table-half).
- xl[src] per-edge rows fetched with dma_gather (int16 idx -> table halves).
- xr[dst] broadcast per edge via one-hot matmul from SBUF-resident xr shard.
- segment softmax denominator + message aggregation via one-hot scatter
  matmul accumulated in PSUM per node block (max-subtraction skipped:
  mathematically identical softmax, alphas are O(5)).
- AllGather (DRAM collective) shares per-shard xl tables between layers.
"""
import numpy as np
import ml_dtypes

import concourse.bass as bass
import concourse.bacc as bacc
import concourse.mybir as mybir
from concourse.tile import TileContext

BF16 = ml_dtypes.bfloat16
F32 = mybir.dt.float32
BF = mybir.dt.bfloat16
I16 = mybir.dt.int16
PAD_DST = 200.0
P = 128
CPC = 8          # chunks per gather call
NI = CPC * 128   # indices per gather call


class Cfg:
    def __init__(self, N, E, nblk, feat=128, heads1=2):
        self.N, self.E = N, E
        self.NBLK = nblk
        self.SHARD = nblk * P
        self.NPAD = 8 * self.SHARD
        assert self.NPAD >= N and self.NPAD % 256 == 0
        self.HALF = self.NPAD // 2
        assert self.HALF <= 32767
        self.F = feat
        self.H1 = heads1
        self.C1 = feat // heads1


def host_prep(cfg, x, edge_index):
    """Returns (per_core_inputs: list of dict, struct: dict)."""
    N, E = cfg.N, cfg.E
    src = np.concatenate([np.asarray(edge_index[0]), np.arange(N)]).astype(np.int64)
    dst = np.concatenate([np.asarray(edge_index[1]), np.arange(N)]).astype(np.int64)
    ET = src.shape[0]

    core = dst // cfg.SHARD
    block = (dst % cfg.SHARD) // P
    dloc = dst % P
    half = (src >= cfg.HALF).astype(np.int64)
    gval = (src - half * cfg.HALF).astype(np.int64)

    # group = (core, half, block); rank within group
    key = (core * 2 + half) * cfg.NBLK + block
    order = np.argsort(key, kind="stable")
    key_s = key[order]
    # counts per group
    ngroups = 8 * 2 * cfg.NBLK
    cnt = np.bincount(key_s, minlength=ngroups)
    starts = np.zeros(ngroups + 1, np.int64)
    np.cumsum(cnt, out=starts[1:])
    rank = np.arange(ET) - starts[key_s]

    cnt3 = cnt.reshape(8, 2, cfg.NBLK)
    S_A = int(np.ceil(cnt3[:, 0, :].max() / P))
    S_B = int(np.ceil(cnt3[:, 1, :].max() / P))
    S_A, S_B = max(S_A, 1), max(S_B, 1)
    CHA = -(-(cfg.NBLK * S_A) // CPC) * CPC
    CHB = -(-(cfg.NBLK * S_B) // CPC) * CPC
    CH = CHA + CHB
    CALLS = CH // CPC

    # chunk -> (half, block) static structure
    chunk_half = np.zeros(CH, np.int64)
    chunk_block = np.zeros(CH, np.int64)
    for c in range(CH):
        if c < CHA:
            chunk_half[c] = 0
            chunk_block[c] = min(c // S_A, cfg.NBLK - 1)
        else:
            chunk_half[c] = 1
            chunk_block[c] = min((c - CHA) // S_B, cfg.NBLK - 1)

    # fill per-core edge arrays
    gidx = np.zeros((8, CH, P), np.int16)
    dstl = np.full((8, CH, P), PAD_DST, np.float32)
    g_half = half[order]
    g_core = core[order]
    g_block = block[order]
    slot_base = np.where(g_half == 0, g_block * S_A, CHA + g_block * S_B)
    slot = slot_base + rank // P
    pos = rank % P
    gidx[g_core, slot, pos] = gval[order].astype(np.int16)
    dstl[g_core, slot, pos] = dloc[order].astype(np.float32)

    # wrap gather indices per call: local idx i -> [i%16, i//16]
    # gidx [8, CALLS, NI] -> per call [NI//16,16].T -> [16, NI//16]
    gw = gidx.reshape(8, CALLS, NI // 16, 16).transpose(0, 1, 3, 2)  # [8,CALLS,16,64]
    gw = gw.transpose(0, 2, 1, 3).reshape(8, 16, CALLS * (NI // 16))
    gw = np.tile(gw, (1, 8, 1))  # replicate to 128 partitions

    struct = dict(S_A=S_A, S_B=S_B, CHA=CHA, CHB=CHB, CH=CH, CALLS=CALLS,
                  chunk_half=chunk_half, chunk_block=chunk_block)

    x_pad = np.zeros((cfg.NPAD, cfg.F), np.float32)
    x_pad[:N] = np.asarray(x, np.float32)

    per_core = []
    for k in range(8):
        per_core.append(dict(
            xTs=np.ascontiguousarray(x_pad[k * cfg.SHARD:(k + 1) * cfg.SHARD].T),
            gidx=np.ascontiguousarray(gw[k]),
            dstl=np.ascontiguousarray(dstl[k].T.astype(BF16)),  # [128, CH]
        ))
    return per_core, struct


def host_consts(cfg, Wl1, Wr1, att1, b1, Wl2, Wr2, att2, b2):
    f = cfg.F
    c = {}
    c["w1"] = np.hstack([np.asarray(Wl1, np.float32), np.asarray(Wr1, np.float32)])
    c["w2"] = np.hstack([np.asarray(Wl2, np.float32), np.asarray(Wr2, np.float32)])
    c["attb1"] = np.tile(np.asarray(att1, np.float32).reshape(1, f), (P, 1)).astype(BF16)
    c["attb2"] = np.tile(np.asarray(att2, np.float32).reshape(1, f), (P, 1)).astype(BF16)
    c["bb1"] = np.tile(np.asarray(b1, np.float32).reshape(1, f), (P, 1))
    c["bb2"] = np.tile(np.asarray(b2, np.float32).reshape(1, f), (P, 1))
    c["iotaF"] = np.tile(np.arange(P, dtype=np.float32).reshape(1, P), (P, 1)).astype(BF16)
    c["iotaP"] = np.tile(np.arange(P, dtype=np.float32).reshape(P, 1), (1, P))
    c["iotaPB"] = c["iotaP"].astype(BF16)
    c["identB"] = np.eye(P, dtype=np.float32).astype(BF16)
    c["identF"] = np.eye(P, dtype=np.float32)
    return c


def _ap(base, layout):
    return bass.AP(base.tensor, base.offset, [list(d) for d in layout])


def build_program(cfg, struct):
    NBLK, SHARD, NPAD, HALF, F = cfg.NBLK, cfg.SHARD, cfg.NPAD, cfg.HALF, cfg.F
    CH, CALLS = struct["CH"], struct["CALLS"]
    chunk_half, chunk_block = struct["chunk_half"], struct["chunk_block"]

    nc = bacc.Bacc("TRN2", target_bir_lowering=False, debug=False,
                   num_devices=8, num_swdge_queues=4)

    # I/O
    xTs = nc.dram_tensor("xTs", [P, SHARD], F32, kind="ExternalInput")
    gidx = nc.dram_tensor("gidx", [P, CALLS * (NI // 16)], I16, kind="ExternalInput")
    dstl = nc.dram_tensor("dstl", [P, CH], BF, kind="ExternalInput")
    w1 = nc.dram_tensor("w1", [P, 2 * F], F32, kind="ExternalInput")
    w2 = nc.dram_tensor("w2", [P, 2 * F], F32, kind="ExternalInput")
    attb1 = nc.dram_tensor("attb1", [P, F], BF, kind="ExternalInput")
    attb2 = nc.dram_tensor("attb2", [P, F], BF, kind="ExternalInput")
    bb1 = nc.dram_tensor("bb1", [P, F], F32, kind="ExternalInput")
    bb2 = nc.dram_tensor("bb2", [P, F], F32, kind="ExternalInput")
    iotaF = nc.dram_tensor("iotaF", [P, P], BF, kind="ExternalInput")
    iotaP = nc.dram_tensor("iotaP", [P, P], F32, kind="ExternalInput")
    iotaPB = nc.dram_tensor("iotaPB", [P, P], BF, kind="ExternalInput")
    identB = nc.dram_tensor("identB", [P, P], BF, kind="ExternalInput")
    identF = nc.dram_tensor("identF", [P, P], F32, kind="ExternalInput")
    out = nc.dram_tensor("out", [SHARD, F], F32, kind="ExternalOutput")

    eq = mybir.AluOpType.is_equal
    mul = mybir.AluOpType.mult
    AF = mybir.ActivationFunctionType
    AX = mybir.AxisListType.X

    with TileContext(nc) as tc:
        with (
            tc.tile_pool(name="const", bufs=1) as cpool,
            tc.tile_pool(name="big", bufs=1) as bigp,
            tc.tile_pool(name="work", bufs=1) as wp,
            tc.tile_pool(name="psum", bufs=1, space="PSUM") as pp,
            tc.tile_pool(name="dram", bufs=1, space="DRAM") as dp,
        ):
            # ---- consts ----
            def load_const(t, shape, dt):
                s = cpool.tile(shape, dt, name=t.name + "_sb")
                nc.sync.dma_start(out=s[:], in_=t[:])
                return s
            w1_sb = load_const(w1, [P, 2 * F], F32)
            w2_sb = load_const(w2, [P, 2 * F], F32)
            attb1_sb = load_const(attb1, [P, F], BF)
            attb2_sb = load_const(attb2, [P, F], BF)
            bb1_sb = load_const(bb1, [P, F], F32)
            bb2_sb = load_const(bb2, [P, F], F32)
            iotaF_sb = load_const(iotaF, [P, P], BF)
            iotaP_sb = load_const(iotaP, [P, P], F32)
            iotaPB_sb = load_const(iotaPB, [P, P], BF)
            identB_sb = load_const(identB, [P, P], BF)
            identF_sb = load_const(identF, [P, P], F32)
            xTs_sb = bigp.tile([P, SHARD], F32, name="xTs_sb", tag="bigshare")
            nc.sync.dma_start(out=xTs_sb[:], in_=xTs[:])
            gidx_sb = bigp.tile([P, CALLS * (NI // 16)], I16, name="gidx_sb")
            nc.sync.dma_start(out=gidx_sb[:], in_=gidx[:])
            dstl_sb = bigp.tile([P, CH], BF, name="dstl_sb")
            nc.sync.dma_start(out=dstl_sb[:], in_=dstl[:])

            # ---- persistent big tiles ----
            lneps_sb = cpool.tile([P, 1], F32, name="lneps_sb")
            nc.vector.memset(lneps_sb[:], float(np.log(1e-16)))
            xr1_sb = bigp.tile([P, SHARD], BF, name="xr1_sb")
            xr2_sb = bigp.tile([P, SHARD], BF, name="xr2_sb")
            hT_sb = bigp.tile([P, SHARD], F32, name="hT_sb", tag="bigshare")
            hacc = bigp.tile([P, NBLK * (F + 4)], F32, name="hacc")
            stage = bigp.tile([P, SHARD], F32, name="stage")  # xl staging / h / out

            # DRAM exchange buffers
            xl1sh = dp.tile([SHARD, F], F32, name="xl1sh")
            xl1full = dp.tile([NPAD, F], F32, name="xl1full", addr_space="Shared")
            xl2sh = dp.tile([SHARD, F], F32, name="xl2sh")
            xl2full = dp.tile([NPAD, F], F32, name="xl2full", addr_space="Shared")

            def shard_matmul(lhs_sb, w_sb, xr_dst, lhs_dt_note):
                """49x: [128n,256] = lhs_blockT.T @ [Wl|Wr]; xl->stage, xr->xr_dst(bf16)."""
                for j in range(NBLK):
                    mm = pp.tile([P, 512], F32, tag="txr", bufs=2, name=f"mm{j}")
                    nc.tensor.matmul(out=mm[:, 0:2 * F],
                                     lhsT=lhs_sb[:, j * P:(j + 1) * P],
                                     rhs=w_sb[:], start=True, stop=True)
                    nc.vector.tensor_copy(out=stage[:, j * F:(j + 1) * F], in_=mm[:, 0:F])
                    nc.vector.tensor_copy(out=xr_dst[:, j * P:(j + 1) * P], in_=mm[:, F:2 * F])

            def dma_stage_to(dram_tile):
                o = dram_tile[:].rearrange("(b p) f -> p b f", p=P)
                i = stage[:].rearrange("p (b f) -> p b f", f=F)
                nc.sync.dma_start(out=o, in_=i)

            # ---- phase 0: xl1/xr1 ----
            shard_matmul(xTs_sb, w1_sb, xr1_sb, "f32")
            dma_stage_to(xl1sh)
            nc.gpsimd.collective_compute(
                "AllGather", mybir.AluOpType.bypass,
                replica_groups=[list(range(8))],
                ins=[xl1sh[:]], outs=[xl1full[:]])

            def edge_pass(layer, table, xr_sb, attb_sb):
                HN = cfg.H1 if layer == 1 else 1
                CW = F // HN
                RW = F + 2 * HN
                nc.vector.memset(hacc[:, 0:NBLK * RW], 0.0)
                bp = None
                for g in range(CALLS):
                    cb = g * CPC
                    hf = int(chunk_half[cb])
                    tab = table[:][0:HALF, :] if hf == 0 else table[:][HALF:NPAD, :]
                    xg = wp.tile([P, CPC, F], F32, tag="xg", bufs=3, name=f"xg{layer}_{g}")
                    nc.gpsimd.dma_gather(
                        out_ap=xg[:], in_ap=tab,
                        idxs_ap=gidx_sb[:, g * (NI // 16):(g + 1) * (NI // 16)],
                        num_idxs=NI, num_idxs_reg=NI, elem_size=F,
                        single_packet=True, queue_num=g % 4)
                    # QT[e, n] one-hot (batched over call)
                    qt = wp.tile([P, CPC, P], BF, tag="qt", bufs=3, name=f"qt{layer}_{g}")
                    d8 = dstl_sb[:, cb:cb + CPC]
                    nc.vector.tensor_tensor(
                        out=qt[:], in0=d8.to_broadcast([P, CPC, P]),
                        in1=_ap(iotaF_sb[:], [iotaF_sb[:].ap[0], [0, CPC], [1, P]]),
                        op=eq)
                    qtf = wp.tile([P, CPC, P], F32, tag="qtf", bufs=2,
                                  name=f"qtf{layer}_{g}")
                    nc.vector.tensor_tensor(
                        out=qtf[:], in0=d8.to_broadcast([P, CPC, P]),
                        in1=_ap(iotaF_sb[:], [iotaF_sb[:].ap[0], [0, CPC], [1, P]]),
                        op=eq)
                    # dstT via PE transpose; Q[n, e] one-hot
                    trp = [pp.tile([P, 512], BF, tag="trp", bufs=2, name=f"trp{layer}_{g}_{i}")
                           for i in range(2)]
                    for c in range(CPC):
                        col = dstl_sb[:, cb + c:cb + c + 1]
                        nc.tensor.transpose(
                            out=trp[c // 4][:, (c % 4) * P:(c % 4 + 1) * P],
                            in_=col.to_broadcast([P, P]), identity=identB_sb[:])
                    q = wp.tile([P, CPC, P], BF, tag="q", bufs=3, name=f"q{layer}_{g}")
                    for i in range(2):
                        nc.vector.tensor_tensor(
                            out=q[:, i * 4:(i + 1) * 4, :],
                            in0=_ap(iotaPB_sb[:], [iotaPB_sb[:].ap[0], [0, 4], [1, P]]),
                            in1=trp[i][:].rearrange("p (c f) -> p c f", f=P),
                            op=eq)
                    # xr gather matmuls
                    txr = [pp.tile([P, 512], F32, tag="txr", bufs=2, name=f"txr{layer}_{g}_{i}")
                           for i in range(2)]
                    for c in range(CPC):
                        blk = int(chunk_block[cb + c])
                        nc.tensor.matmul(
                            out=txr[c // 4][:, (c % 4) * P:(c % 4 + 1) * P],
                            lhsT=q[:, c, :], rhs=xr_sb[:, blk * P:(blk + 1) * P],
                            start=True, stop=True)
                    # t = xg + xr
                    tt = wp.tile([P, NI], F32, tag="tt", bufs=2, name=f"tt{layer}_{g}")
                    for i in range(2):
                        nc.vector.tensor_add(
                            out=tt[:, i * 512:(i + 1) * 512],
                            in0=xg[:, i * 4:(i + 1) * 4, :].rearrange("p c f -> p (c f)"),
                            in1=txr[i][:])
                    # leaky relu in one ACT pass (Prelu: x>=0 ? x : alpha*x)
                    lr = wp.tile([P, NI], BF, tag="lra", bufs=2, name=f"lra{layer}_{g}")
                    nc.scalar.activation(out=lr[:], in_=tt[:], func=AF.Prelu,
                                         alpha=0.2)
                    # u = lr * att
                    u = wp.tile([P, NI], F32, tag="u", bufs=2, name=f"u{layer}_{g}")
                    nc.vector.tensor_tensor(
                        out=u[:].rearrange("p (c f) -> p c f", f=F),
                        in0=lr[:].rearrange("p (c f) -> p c f", f=F),
                        in1=_ap(attb_sb[:], [attb_sb[:].ap[0], [0, CPC], [1, F]]),
                        op=mul)
                    # alpha, p
                    al = wp.tile([P, CPC * HN], F32, tag="al", bufs=2, name=f"al{layer}_{g}")
                    nc.vector.reduce_sum(
                        out=al[:], in_=u[:].rearrange("p (g s) -> p g s", s=CW), axis=AX)
                    pe = wp.tile([P, CPC * HN], F32, tag="pe", bufs=2, name=f"pe{layer}_{g}")
                    nc.scalar.activation(out=pe[:], in_=al[:], func=AF.Exp)
                    # rhs = [p * xg | p]
                    RWB = F + HN
                    rhs = wp.tile([P, CPC * RWB], BF, tag="rhs", bufs=3, name=f"rhs{layer}_{g}")
                    rbase = rhs[:]
                    pbase = pe[:]
                    xbase = xg[:]
                    nc.vector.tensor_tensor(
                        out=_ap(rbase, [rbase.ap[0], [RWB, CPC], [CW, HN], [1, CW]]),
                        in0=_ap(xbase, [xbase.ap[0], [F, CPC], [CW, HN], [1, CW]]),
                        in1=_ap(pbase, [pbase.ap[0], [HN, CPC], [1, HN], [0, CW]]),
                        op=mul)
                    pc_out = bass.AP(rbase.tensor, rbase.offset + F,
                                     [list(rbase.ap[0]), [RWB, CPC], [1, HN]])
                    nc.vector.tensor_copy(
                        out=pc_out, in_=pbase.rearrange("p (c h) -> p c h", h=HN))
                    # scatter matmuls per chunk, PSUM-accumulated per block segment
                    for c in range(CPC):
                        ci = cb + c
                        blk = int(chunk_block[ci])
                        seg_start = ci == 0 or chunk_block[ci - 1] != blk
                        seg_end = ci == CH - 1 or chunk_block[ci + 1] != blk
                        if seg_start:
                            bp = pp.tile([P, RWB], F32, tag="bp", bufs=2, name=f"bp{layer}_{ci}")
                            ba = pp.tile([P, 2 * HN], F32, tag="ba", bufs=2, name=f"ba{layer}_{ci}")
                        nc.tensor.matmul(
                            out=bp[:], lhsT=qt[:, c, :],
                            rhs=rhs[:, c * RWB:(c + 1) * RWB],
                            start=seg_start, stop=seg_end)
                        nc.tensor.matmul(
                            out=ba[:, 0:HN], lhsT=qtf[:, c, :],
                            rhs=al[:, c * HN:(c + 1) * HN],
                            start=seg_start, stop=seg_end)
                        if seg_end:
                            nc.vector.tensor_add(
                                out=hacc[:, blk * RW:blk * RW + RWB],
                                in0=hacc[:, blk * RW:blk * RW + RWB], in1=bp[:])
                            nc.vector.tensor_add(
                                out=hacc[:, blk * RW + RWB:blk * RW + RW],
                                in0=hacc[:, blk * RW + RWB:blk * RW + RW], in1=ba[:, 0:HN])

            def elu_inplace(sl, tmp1, tmp2):
                nc.vector.tensor_scalar_min(out=tmp1[:], in0=sl, scalar1=0.0)
                nc.scalar.activation(out=tmp2[:], in_=tmp1[:], func=AF.Exp)
                nc.vector.tensor_scalar_max(out=sl, in0=sl, scalar1=0.0)
                nc.vector.tensor_add(out=sl, in0=sl, in1=tmp2[:])
                nc.vector.tensor_scalar_add(out=sl, in0=sl, scalar1=-1.0)

            def epilogue(layer, bb_sb):
                HN = cfg.H1 if layer == 1 else 1
                CW = F // HN
                RW = F + 2 * HN
                LN_EPS = float(np.log(1e-16))
                for b in range(NBLK):
                    eps = wp.tile([P, HN], F32, tag="eps", bufs=2, name=f"eps{layer}_{b}")
                    nc.scalar.activation(
                        out=eps[:], in_=hacc[:, b * RW + F + HN:b * RW + RW],
                        func=AF.Exp, bias=lneps_sb[:, 0:1])
                    den = wp.tile([P, HN], F32, tag="den", bufs=2, name=f"den{layer}_{b}")
                    nc.vector.tensor_add(
                        out=den[:], in0=hacc[:, b * RW + F:b * RW + F + HN], in1=eps[:])
                    rec = wp.tile([P, HN], F32, tag="rec", bufs=2, name=f"rec{layer}_{b}")
                    nc.vector.reciprocal(out=rec[:], in_=den[:])
                    sl = stage[:, b * F:(b + 1) * F]
                    for h in range(HN):
                        nc.vector.tensor_tensor(
                            out=stage[:, b * F + h * CW:b * F + (h + 1) * CW],
                            in0=hacc[:, b * RW + h * CW:b * RW + (h + 1) * CW],
                            in1=rec[:, h:h + 1].to_broadcast([P, CW]), op=mul)
                    nc.vector.tensor_add(out=sl, in0=sl, in1=bb_sb[:])
                    tmp1 = wp.tile([P, F], F32, tag="tmp1", bufs=2, name=f"t1_{layer}_{b}")
                    tmp2 = wp.tile([P, F], F32, tag="tmp2", bufs=2, name=f"t2_{layer}_{b}")
                    elu_inplace(sl, tmp1, tmp2)
                    if layer == 1:
                        trh = pp.tile([P, 512], F32, tag="trp", bufs=2, name=f"trh{b}")
                        nc.tensor.transpose(out=trh[:, 0:P], in_=sl, identity=identF_sb[:])
                        nc.vector.tensor_copy(out=hT_sb[:, b * P:(b + 1) * P], in_=trh[:, 0:P])

            # ---- layer 1 ----
            edge_pass(1, xl1full, xr1_sb, attb1_sb)
            epilogue(1, bb1_sb)
            # ---- phase 2: xl2/xr2 from hT ----
            shard_matmul(hT_sb, w2_sb, xr2_sb, "f32")
            dma_stage_to(xl2sh)
            nc.gpsimd.collective_compute(
                "AllGather", mybir.AluOpType.bypass,
                replica_groups=[list(range(8))],
                ins=[xl2sh[:]], outs=[xl2full[:]])
            # ---- layer 2 ----
            edge_pass(2, xl2full, xr2_sb, attb2_sb)
            epilogue(2, bb2_sb)
            # write out
            oo = out[:].rearrange("(b p) f -> p b f", p=P)
            ii = stage[:].rearrange("p (b f) -> p b f", f=F)
            nc.sync.dma_start(out=oo, in_=ii)

    nc.compile()
    return nc


def run(cfg, inputs, trace=False, core_results=False):
    from concourse.bass_utils import run_bass_kernel_spmd
    x = np.asarray(inputs["x"], np.float32)
    ei = np.asarray(inputs["edge_index"])
    per_core, struct = host_prep(cfg, x, ei)
    consts = host_consts(cfg, *[inputs[k] for k in
                                ("Wl1", "Wr1", "att1", "b1", "Wl2", "Wr2", "att2", "b2")])
    nc = build_program(cfg, struct)
    in_maps = []
    for k in range(8):
        m = dict(per_core[k])
        m.update(consts)
        in_maps.append(m)
    res = run_bass_kernel_spmd(nc, in_maps, core_ids=list(range(8)), trace=trace)
    outs = [res.results[k]["out"] for k in range(8)]
    full = np.concatenate(outs, axis=0)[:cfg.N]
    return full, res


# ---------------------------------------------------------------------------
# public entry point
# ---------------------------------------------------------------------------
_CACHE = {}
LAST_RESULTS = None


def _trace_enabled():
    import os
    return os.environ.get("GAT_TRACE", "") == "1"


def _install_trace_shim():
    """antenv.axon_hooks is absent in this image; recreate it so trace=True
    can capture NTFF profiles through the axon PJRT plugin."""
    import sys, types
    if "antenv.axon_hooks" in sys.modules:
        return
    try:
        mod = types.ModuleType("antenv.axon_hooks")
        mod._hook = None
        mod.set_axon_ntff_profile_hook = lambda h: setattr(mod, "_hook", h)
        mod.get_axon_ntff_profile_hook = lambda: mod._hook
        sys.modules["antenv.axon_hooks"] = mod
        import antenv
        antenv.axon_hooks = mod
        from trn_agent_boot.trn_boot import _ntff_profile_via_ctypes
        mod._hook = _ntff_profile_via_ctypes("/opt/axon/libaxon_pjrt.so")
        import concourse.bass_utils as bu
        bu.upload_artifacts = lambda tmpdir: str(tmpdir)
    except Exception:
        pass


def kernel(x, edge_index, Wl1, Wr1, att1, b1, Wl2, Wr2, att2, b2):
    global LAST_RESULTS
    from concourse.bass_utils import run_bass_kernel_spmd

    trace = _trace_enabled()
    if trace:
        _install_trace_shim()

    x = np.asarray(x, np.float32)
    edge_index = np.asarray(edge_index)
    N, E = x.shape[0], edge_index.shape[1]
    cfg = Cfg(N, E, nblk=49)

    per_core, struct = host_prep(cfg, x, edge_index)
    consts = host_consts(cfg, Wl1, Wr1, att1, b1, Wl2, Wr2, att2, b2)

    key = (N, E, x.shape[1], struct["S_A"], struct["S_B"])
    if key not in _CACHE:
        _CACHE[key] = build_program(cfg, struct)
    nc = _CACHE[key]

    in_maps = []
    for k in range(8):
        m = dict(per_core[k])
        m.update(consts)
        in_maps.append(m)
    res = run_bass_kernel_spmd(nc, in_maps, core_ids=list(range(8)), trace=trace)
    LAST_RESULTS = res
    outs = [np.asarray(res.results[k]["out"]) for k in range(8)]
    return np.concatenate(outs, axis=0)[:N].astype(np.float32)



# revision 16
# speedup vs baseline: 1.2012x; 1.2012x over previous
"""Trainium2 Bass kernel for a 2-layer GATv2 aggregator (N=50000, E=800000).

Self-contained: kernel(**inputs) takes full inputs, shards across 8
NeuronCores internally, returns the full (50000, 128) float32 output.

v2 strategy (8-core SPMD, dst-sharded):
- Channels permuted per head (positive-att first) and tables pre-scaled by the
  SIGNED att value: t~_c = att_c*(xl_c + xr_c).  Then
  att_c*leaky(t_c) = Prelu(t~_c; 0.2) for att_c>0 and min(t~, 0.2 t~)
  = Prelu(0.2*t~; 5) for att_c<0, so alpha = plain per-head sum of the
  ACT output -- one strided reduce, no per-edge att multiply.
- Messages aggregate Sum p*x~l (scaled); epilogue divides by att_c per
  channel (recip const tile).  Layer-2 weight rows pre-permuted; final
  output unpermuted on host.
- All edge-pass tiles bf16 (tables, one-hots, rhs);  eps term
  1e-16*exp(sum alpha) (replicates the oracle's segment_max-is-sum bug)
  accumulated via exact hi/lo bf16 split columns in the scatter matmul.
- dma_gather with prepare_only+trigger_dma so SWDGE desc-gen overlaps
  the DMA drain;  gather calls of 2048 edges (bf16 rows, 256 B).
"""
import numpy as np
import ml_dtypes

import concourse.bass as bass
import concourse.bacc as bacc
import concourse.mybir as mybir
from concourse.tile import TileContext

BF16 = ml_dtypes.bfloat16
F32 = mybir.dt.float32
BF = mybir.dt.bfloat16
I16 = mybir.dt.int16
PAD_DST = 200.0
P = 128
CPC = 8           # chunks per compute group
GPC = 8           # chunks per gather call (1024 idx = SWDGE ring max)
NI = GPC * 128    # indices per gather call
import os
USE_PREP = os.environ.get("GAT_PREP", "0") == "1"


class Cfg:
    def __init__(self, N, E, nblk, feat=128, heads1=2):
        self.N, self.E = N, E
        self.NBLK = nblk
        self.SHARD = nblk * P
        self.NPAD = 8 * self.SHARD
        assert self.NPAD >= N and self.NPAD % 256 == 0
        self.HALF = self.NPAD // 2
        assert self.HALF <= 32767
        self.F = feat
        self.H1 = heads1
        self.C1 = feat // heads1


def host_prep(cfg, x, edge_index):
    """Returns (per_core_inputs: list of dict, struct: dict)."""
    N, E = cfg.N, cfg.E
    src = np.concatenate([np.asarray(edge_index[0]), np.arange(N)]).astype(np.int64)
    dst = np.concatenate([np.asarray(edge_index[1]), np.arange(N)]).astype(np.int64)
    ET = src.shape[0]

    core = dst // cfg.SHARD
    block = (dst % cfg.SHARD) // P
    dloc = dst % P
    half = (src >= cfg.HALF).astype(np.int64)
    gval = (src - half * cfg.HALF).astype(np.int64)

    # group = (core, half, block); rank within group
    key = (core * 2 + half) * cfg.NBLK + block
    order = np.argsort(key, kind="stable")
    key_s = key[order]
    ngroups = 8 * 2 * cfg.NBLK
    cnt = np.bincount(key_s, minlength=ngroups)
    starts = np.zeros(ngroups + 1, np.int64)
    np.cumsum(cnt, out=starts[1:])
    rank = np.arange(ET) - starts[key_s]

    cnt3 = cnt.reshape(8, 2, cfg.NBLK)
    S_A = int(np.ceil(cnt3[:, 0, :].max() / P))
    S_B = int(np.ceil(cnt3[:, 1, :].max() / P))
    S_A, S_B = max(S_A, 1), max(S_B, 1)
    CHA = -(-(cfg.NBLK * S_A) // GPC) * GPC
    CHB = -(-(cfg.NBLK * S_B) // GPC) * GPC
    CH = CHA + CHB
    CALLS = CH // GPC

    chunk_half = np.zeros(CH, np.int64)
    chunk_block = np.zeros(CH, np.int64)
    for c in range(CH):
        if c < CHA:
            chunk_half[c] = 0
            chunk_block[c] = min(c // S_A, cfg.NBLK - 1)
        else:
            chunk_half[c] = 1
            chunk_block[c] = min((c - CHA) // S_B, cfg.NBLK - 1)

    gidx = np.zeros((8, CH, P), np.int16)
    dstl = np.full((8, CH, P), PAD_DST, np.float32)
    g_half = half[order]
    g_core = core[order]
    g_block = block[order]
    slot_base = np.where(g_half == 0, g_block * S_A, CHA + g_block * S_B)
    slot = slot_base + rank // P
    pos = rank % P
    gidx[g_core, slot, pos] = gval[order].astype(np.int16)
    dstl[g_core, slot, pos] = dloc[order].astype(np.float32)

    # wrap gather indices per call of NI: [NI//16,16].T -> [16, NI//16]
    gw = gidx.reshape(8, CALLS, NI // 16, 16).transpose(0, 1, 3, 2)
    gw = gw.transpose(0, 2, 1, 3).reshape(8, 16, CALLS * (NI // 16))
    gw = np.tile(gw, (1, 8, 1))  # replicate to 128 partitions

    struct = dict(S_A=S_A, S_B=S_B, CHA=CHA, CHB=CHB, CH=CH, CALLS=CALLS,
                  chunk_half=chunk_half, chunk_block=chunk_block)

    x_pad = np.zeros((cfg.NPAD, cfg.F), np.float32)
    x_pad[:N] = np.asarray(x, np.float32)

    per_core = []
    for k in range(8):
        per_core.append(dict(
            xTs=np.ascontiguousarray(
                x_pad[k * cfg.SHARD:(k + 1) * cfg.SHARD].T.astype(BF16)),
            gidx=np.ascontiguousarray(gw[k]),
            dstl=np.ascontiguousarray(dstl[k].T.astype(BF16)),  # [128, CH]
        ))
    return per_core, struct


def _perm_layer(Wl, Wr, att):
    """Channel perm (positive att first per head) + signed-scale weights."""
    att = np.asarray(att, np.float32)
    H, C = att.shape
    perm = np.zeros((H, C), np.int64)
    k = np.zeros(H, np.int64)
    for h in range(H):
        pos = np.where(att[h] > 0)[0]
        neg = np.where(att[h] <= 0)[0]
        perm[h] = np.concatenate([pos, neg])
        k[h] = len(pos)
    att_p = np.take_along_axis(att, perm, axis=1)
    s = att_p.reshape(-1)                    # signed scale per (permuted) chan
    flat_perm = (perm + np.arange(H)[:, None] * C).reshape(-1)
    Wl_t = np.asarray(Wl, np.float32)[:, flat_perm] * s[None, :]
    Wr_t = np.asarray(Wr, np.float32)[:, flat_perm] * s[None, :]
    return Wl_t, Wr_t, s, k, flat_perm


def host_consts(cfg, Wl1, Wr1, att1, b1, Wl2, Wr2, att2, b2):
    f = cfg.F
    Wl1t, Wr1t, s1, k1, perm1 = _perm_layer(Wl1, Wr1, att1)
    # layer2 rows permuted by perm1 (its input h is in permuted-1 order)
    Wl2t, Wr2t, s2, k2, perm2 = _perm_layer(
        np.asarray(Wl2, np.float32)[perm1], np.asarray(Wr2, np.float32)[perm1],
        att2)
    c = {}
    c["w1"] = np.hstack([Wl1t, Wr1t]).astype(BF16)
    c["w2"] = np.hstack([Wl2t, Wr2t]).astype(BF16)
    c["recip1"] = np.tile((1.0 / s1).reshape(1, f), (P, 1)).astype(np.float32)
    c["recip2"] = np.tile((1.0 / s2).reshape(1, f), (P, 1)).astype(np.float32)
    c["bb1"] = np.tile(np.asarray(b1, np.float32)[perm1].reshape(1, f), (P, 1))
    c["bb2"] = np.tile(np.asarray(b2, np.float32)[perm2].reshape(1, f), (P, 1))
    c["iotaF"] = np.tile(np.arange(P, dtype=np.float32).reshape(1, P),
                         (P, 1)).astype(BF16)
    c["iotaPB"] = np.tile(np.arange(P, dtype=np.float32).reshape(P, 1), (1, P)
                          ).astype(BF16)
    c["identB"] = np.eye(P, dtype=np.float32).astype(BF16)
    c["identF"] = np.eye(P, dtype=np.float32)
    meta = dict(k1=tuple(int(v) for v in k1), k2=tuple(int(v) for v in k2),
                perm2=perm2)
    return c, meta


def _ap(base, layout):
    return bass.AP(base.tensor, base.offset, [list(d) for d in layout])


def build_program(cfg, struct, k1, k2):
    NBLK, SHARD, NPAD, HALF, F = cfg.NBLK, cfg.SHARD, cfg.NPAD, cfg.HALF, cfg.F
    CH, CALLS = struct["CH"], struct["CALLS"]
    chunk_half, chunk_block = struct["chunk_half"], struct["chunk_block"]
    RW = F + 8

    nc = bacc.Bacc("TRN2", target_bir_lowering=False, debug=False,
                   num_devices=8, num_swdge_queues=4)

    xTs = nc.dram_tensor("xTs", [P, SHARD], BF, kind="ExternalInput")
    gidx = nc.dram_tensor("gidx", [P, CALLS * (NI // 16)], I16, kind="ExternalInput")
    dstl = nc.dram_tensor("dstl", [P, CH], BF, kind="ExternalInput")
    w1 = nc.dram_tensor("w1", [P, 2 * F], BF, kind="ExternalInput")
    w2 = nc.dram_tensor("w2", [P, 2 * F], BF, kind="ExternalInput")
    recip1 = nc.dram_tensor("recip1", [P, F], F32, kind="ExternalInput")
    recip2 = nc.dram_tensor("recip2", [P, F], F32, kind="ExternalInput")
    bb1 = nc.dram_tensor("bb1", [P, F], F32, kind="ExternalInput")
    bb2 = nc.dram_tensor("bb2", [P, F], F32, kind="ExternalInput")
    iotaF = nc.dram_tensor("iotaF", [P, P], BF, kind="ExternalInput")
    iotaPB = nc.dram_tensor("iotaPB", [P, P], BF, kind="ExternalInput")
    identB = nc.dram_tensor("identB", [P, P], BF, kind="ExternalInput")
    identF = nc.dram_tensor("identF", [P, P], F32, kind="ExternalInput")
    out = nc.dram_tensor("out", [SHARD, F], F32, kind="ExternalOutput")

    eq = mybir.AluOpType.is_equal
    mul = mybir.AluOpType.mult
    AF = mybir.ActivationFunctionType
    AX = mybir.AxisListType.X

    with TileContext(nc) as tc:
        with (
            tc.tile_pool(name="const", bufs=1) as cpool,
            tc.tile_pool(name="big", bufs=1) as bigp,
            tc.tile_pool(name="work", bufs=1) as wp,
            tc.tile_pool(name="psum", bufs=1, space="PSUM") as pp,
            tc.tile_pool(name="dram", bufs=1, space="DRAM") as dp,
        ):
            def load_const(t, shape, dt):
                s = cpool.tile(shape, dt, name=t.name + "_sb")
                nc.sync.dma_start(out=s[:], in_=t[:])
                return s
            w1_sb = load_const(w1, [P, 2 * F], BF)
            w2_sb = load_const(w2, [P, 2 * F], BF)
            recip1_sb = load_const(recip1, [P, F], F32)
            recip2_sb = load_const(recip2, [P, F], F32)
            bb1_sb = load_const(bb1, [P, F], F32)
            bb2_sb = load_const(bb2, [P, F], F32)
            iotaF_sb = load_const(iotaF, [P, P], BF)
            iotaPB_sb = load_const(iotaPB, [P, P], BF)
            identB_sb = load_const(identB, [P, P], BF)
            identF_sb = load_const(identF, [P, P], F32)
            xTs_sb = bigp.tile([P, SHARD], BF, name="xTs_sb")
            nc.sync.dma_start(out=xTs_sb[:], in_=xTs[:])
            gidx_sb = bigp.tile([P, CALLS * (NI // 16)], I16, name="gidx_sb")
            nc.sync.dma_start(out=gidx_sb[:], in_=gidx[:])
            dstl_sb = bigp.tile([P, CH], BF, name="dstl_sb")
            nc.sync.dma_start(out=dstl_sb[:], in_=dstl[:])

            lneps_sb = cpool.tile([P, 1], F32, name="lneps_sb")
            nc.vector.memset(lneps_sb[:], float(np.log(1e-16)))
            xr1_sb = bigp.tile([P, SHARD], BF, name="xr1_sb")
            xr2_sb = bigp.tile([P, SHARD], BF, name="xr2_sb")
            hT_sb = bigp.tile([P, SHARD], BF, name="hT_sb")
            hacc = bigp.tile([P, NBLK * RW], F32, name="hacc")
            stage = bigp.tile([P, SHARD], BF, name="stage")      # xl (bf16)
            stage_o = bigp.tile([P, SHARD], F32, name="stage_o")  # epilogue f32

            xl1sh = dp.tile([SHARD, F], BF, name="xl1sh")
            xl1full = dp.tile([NPAD, F], BF, name="xl1full", addr_space="Shared")
            xl2sh = dp.tile([SHARD, F], BF, name="xl2sh")
            xl2full = dp.tile([NPAD, F], BF, name="xl2full", addr_space="Shared")

            dma_sem = nc.alloc_semaphore("gat_dma")

            def node_phase(src_sb, w_sb, xr_dst):
                for j in range(NBLK):
                    mm = pp.tile([P, 8, P], F32, tag="txr", bufs=2, name=f"mm{j}")
                    mf = mm[:].rearrange("p c f -> p (c f)")
                    nc.tensor.matmul(out=mf[:, 0:2 * F],
                                     lhsT=src_sb[:, j * P:(j + 1) * P],
                                     rhs=w_sb[:], start=True, stop=True)
                    nc.vector.tensor_copy(out=stage[:, j * F:(j + 1) * F],
                                          in_=mf[:, 0:F])
                    nc.vector.tensor_copy(out=xr_dst[:, j * P:(j + 1) * P],
                                          in_=mf[:, F:2 * F])

            def dma_stage_to(dram_tile):
                o = dram_tile[:].rearrange("(b p) f -> p b f", p=P)
                i = stage[:].rearrange("p (b f) -> p b f", f=F)
                nc.sync.dma_start(out=o, in_=i)

            def edge_pass(layer, table, xr_sb, ks):
                HN = cfg.H1 if layer == 1 else 1
                CW = F // HN
                RWB = F + 3 * HN
                nc.vector.memset(hacc[:], 0.0)
                bp = None
                for g in range(CALLS):
                    cb0 = g * GPC
                    hf = int(chunk_half[cb0])
                    tab = table[:][0:HALF, :] if hf == 0 else table[:][HALF:NPAD, :]
                    xg = wp.tile([P, GPC, F], BF, tag="xg", bufs=3,
                                 name=f"xg{layer}_{g}")
                    if USE_PREP:
                        nc.gpsimd.dma_gather(
                            out_ap=xg[:], in_ap=tab,
                            idxs_ap=gidx_sb[:, g * (NI // 16):(g + 1) * (NI // 16)],
                            num_idxs=NI, num_idxs_reg=NI, elem_size=F,
                            prepare_only=True, sem=dma_sem, queue_num=0)
                        nc.gpsimd.trigger_dma(count=None)
                    else:
                        nc.gpsimd.dma_gather(
                            out_ap=xg[:], in_ap=tab,
                            idxs_ap=gidx_sb[:, g * (NI // 16):(g + 1) * (NI // 16)],
                            num_idxs=NI, num_idxs_reg=NI, elem_size=F,
                            queue_num=g % 4)
                    for sub in range(GPC // CPC):
                        cb = cb0 + sub * CPC
                        d8 = dstl_sb[:, cb:cb + CPC]
                        # one-hot QT[e, n] (lhsT for scatter)
                        qt = wp.tile([P, CPC, P], BF, tag="qt", bufs=3,
                                     name=f"qt{layer}_{cb}")
                        nc.vector.tensor_tensor(
                            out=qt[:], in0=d8.to_broadcast([P, CPC, P]),
                            in1=_ap(iotaF_sb[:],
                                    [iotaF_sb[:].ap[0], [0, CPC], [1, P]]),
                            op=eq)
                        # dstT broadcast via PE transpose; Q[n, e] one-hot
                        trp = pp.tile([P, CPC * P], BF, tag="trp", bufs=2,
                                      name=f"trp{layer}_{cb}")
                        for c in range(CPC):
                            col = dstl_sb[:, cb + c:cb + c + 1]
                            nc.tensor.transpose(
                                out=trp[:, c * P:(c + 1) * P],
                                in_=col.to_broadcast([P, P]),
                                identity=identB_sb[:])
                        q = wp.tile([P, CPC, P], BF, tag="q", bufs=3,
                                    name=f"q{layer}_{cb}")
                        nc.vector.tensor_tensor(
                            out=q[:],
                            in0=_ap(iotaPB_sb[:],
                                    [iotaPB_sb[:].ap[0], [0, CPC], [1, P]]),
                            in1=trp[:].rearrange("p (c f) -> p c f", f=P),
                            op=eq)
                        # xr gather matmuls
                        txr = pp.tile([P, CPC, P], F32, tag="txr", bufs=2,
                                      name=f"txr{layer}_{cb}")
                        for c in range(CPC):
                            blk = int(chunk_block[cb + c])
                            nc.tensor.matmul(
                                out=txr[:, c, :], lhsT=q[:, c, :],
                                rhs=xr_sb[:, blk * P:(blk + 1) * P],
                                start=True, stop=True)
                        # t~ = xg + txr  (bf16 out)
                        tt = wp.tile([P, CPC, F], BF, tag="tt", bufs=2,
                                     name=f"tt{layer}_{cb}")
                        nc.vector.tensor_add(
                            out=tt[:], in0=xg[:, sub * CPC:(sub + 1) * CPC, :],
                            in1=txr[:])
                        # att_c*leaky(t_c): Prelu(x;.2) pos block,
                        # Prelu(.2x;5)=min(x,.2x) neg block, per head
                        lr = wp.tile([P, CPC, F], BF, tag="lr", bufs=2,
                                     name=f"lr{layer}_{cb}")
                        tb = tt[:]
                        lb = lr[:]
                        for h in range(HN):
                            kh = ks[h]
                            if kh > 0:
                                ap_i = bass.AP(tb.tensor, tb.offset + h * CW,
                                               [list(tb.ap[0]), [F, CPC], [1, kh]])
                                ap_o = bass.AP(lb.tensor, lb.offset + h * CW,
                                               [list(lb.ap[0]), [F, CPC], [1, kh]])
                                nc.scalar.activation(out=ap_o, in_=ap_i,
                                                     func=AF.Prelu, alpha=0.2)
                            if kh < CW:
                                off = h * CW + kh
                                ap_i = bass.AP(tb.tensor, tb.offset + off,
                                               [list(tb.ap[0]), [F, CPC],
                                                [1, CW - kh]])
                                ap_o = bass.AP(lb.tensor, lb.offset + off,
                                               [list(lb.ap[0]), [F, CPC],
                                                [1, CW - kh]])
                                nc.scalar.activation(out=ap_o, in_=ap_i,
                                                     func=AF.Prelu, alpha=5.0,
                                                     scale=0.2)
                        # alpha = per-head sum
                        al = wp.tile([P, CPC * HN], F32, tag="al", bufs=2,
                                     name=f"al{layer}_{cb}")
                        nc.vector.reduce_sum(
                            out=al[:],
                            in_=lr[:].rearrange("p c (h s) -> p c h s", s=CW),
                            axis=AX)
                        # exact hi/lo split of alpha for the eps term
                        thi = wp.tile([P, CPC * HN], BF, tag="thi", bufs=2,
                                      name=f"thi{layer}_{cb}")
                        nc.vector.tensor_scalar_add(out=thi[:], in0=al[:],
                                                    scalar1=8.0)
                        ahl = wp.tile([P, CPC, 2 * HN], BF, tag="ahl", bufs=2,
                                      name=f"ahl{layer}_{cb}")
                        nc.vector.tensor_scalar_add(
                            out=ahl[:, :, 0:HN], in0=thi[:].rearrange(
                                "p (c h) -> p c h", h=HN), scalar1=-8.0)
                        nc.vector.tensor_tensor(
                            out=ahl[:, :, HN:2 * HN],
                            in0=al[:].rearrange("p (c h) -> p c h", h=HN),
                            in1=ahl[:, :, 0:HN],
                            op=mybir.AluOpType.subtract)
                        # p = exp(alpha) (bf16)
                        pe = wp.tile([P, CPC * HN], BF, tag="pe", bufs=2,
                                     name=f"pe{layer}_{cb}")
                        nc.scalar.activation(out=pe[:], in_=al[:], func=AF.Exp)
                        # rhs = [p*xg | p | hi | lo]
                        rhs = wp.tile([P, CPC, RWB], BF, tag="rhs", bufs=3,
                                      name=f"rhs{layer}_{cb}")
                        rb = rhs[:]
                        xb = xg[:, sub * CPC:(sub + 1) * CPC, :]
                        pb = pe[:]
                        nc.vector.tensor_tensor(
                            out=_ap(rb, [rb.ap[0], [RWB, CPC], [CW, HN], [1, CW]]),
                            in0=_ap(xb, [xb.ap[0], [F, CPC], [CW, HN], [1, CW]]),
                            in1=_ap(pb, [pb.ap[0], [HN, CPC], [1, HN], [0, CW]]),
                            op=mul)
                        pc_out = bass.AP(rb.tensor, rb.offset + F,
                                         [list(rb.ap[0]), [RWB, CPC], [1, HN]])
                        nc.vector.tensor_copy(
                            out=pc_out, in_=pb.rearrange("p (c h) -> p c h", h=HN))
                        hl_out = bass.AP(rb.tensor, rb.offset + F + HN,
                                         [list(rb.ap[0]), [RWB, CPC], [1, 2 * HN]])
                        nc.vector.tensor_copy(out=hl_out, in_=ahl[:])
                        # scatter matmuls, PSUM-accumulated per block segment
                        for c in range(CPC):
                            ci = cb + c
                            blk = int(chunk_block[ci])
                            seg_start = ci == 0 or chunk_block[ci - 1] != blk
                            seg_end = ci == CH - 1 or chunk_block[ci + 1] != blk
                            if seg_start:
                                bp = pp.tile([P, RWB], F32, tag="bp", bufs=2,
                                             name=f"bp{layer}_{ci}")
                            nc.tensor.matmul(
                                out=bp[:], lhsT=qt[:, c, :], rhs=rhs[:, c, :],
                                start=seg_start, stop=seg_end)
                            if seg_end:
                                nc.vector.tensor_add(
                                    out=hacc[:, blk * RW:blk * RW + RWB],
                                    in0=hacc[:, blk * RW:blk * RW + RWB],
                                    in1=bp[:])

            def elu_inplace(sl, tmp1, tmp2):
                nc.vector.tensor_scalar_min(out=tmp1[:], in0=sl, scalar1=0.0)
                nc.scalar.activation(out=tmp2[:], in_=tmp1[:], func=AF.Exp)
                nc.vector.tensor_scalar_max(out=sl, in0=sl, scalar1=0.0)
                nc.vector.tensor_add(out=sl, in0=sl, in1=tmp2[:])
                nc.vector.tensor_scalar_add(out=sl, in0=sl, scalar1=-1.0)

            def epilogue(layer, recip_sb, bb_sb):
                HN = cfg.H1 if layer == 1 else 1
                CW = F // HN
                RWB = F + 3 * HN
                for b in range(NBLK):
                    hb = hacc[:, b * RW:b * RW + RWB]
                    sa = wp.tile([P, HN], F32, tag="sa", bufs=2,
                                 name=f"sa{layer}_{b}")
                    nc.vector.tensor_add(
                        out=sa[:], in0=hacc[:, b * RW + F + HN:b * RW + F + 2 * HN],
                        in1=hacc[:, b * RW + F + 2 * HN:b * RW + F + 3 * HN])
                    eps = wp.tile([P, HN], F32, tag="eps", bufs=2,
                                  name=f"eps{layer}_{b}")
                    nc.scalar.activation(out=eps[:], in_=sa[:], func=AF.Exp,
                                         bias=lneps_sb[:, 0:1])
                    den = wp.tile([P, HN], F32, tag="den", bufs=2,
                                  name=f"den{layer}_{b}")
                    nc.vector.tensor_add(
                        out=den[:], in0=hacc[:, b * RW + F:b * RW + F + HN],
                        in1=eps[:])
                    rec = wp.tile([P, HN], F32, tag="rec", bufs=2,
                                  name=f"rec{layer}_{b}")
                    nc.vector.reciprocal(out=rec[:], in_=den[:])
                    # scale[n, c] = rec[n, head(c)] * recip_att[c]
                    sc = wp.tile([P, F], F32, tag="sc", bufs=2,
                                 name=f"sc{layer}_{b}")
                    rb = rec[:]
                    nc.vector.tensor_tensor(
                        out=sc[:].rearrange("p (h s) -> p h s", s=CW),
                        in0=_ap(rb, [rb.ap[0], [1, HN], [0, CW]]),
                        in1=recip_sb[:].rearrange("p (h s) -> p h s", s=CW),
                        op=mul)
                    sl = stage_o[:, b * F:(b + 1) * F]
                    nc.vector.tensor_tensor(out=sl, in0=hacc[:, b * RW:b * RW + F],
                                            in1=sc[:], op=mul)
                    nc.vector.tensor_add(out=sl, in0=sl, in1=bb_sb[:])
                    tmp1 = wp.tile([P, F], F32, tag="tmp1", bufs=2,
                                   name=f"t1_{layer}_{b}")
                    tmp2 = wp.tile([P, F], F32, tag="tmp2", bufs=2,
                                   name=f"t2_{layer}_{b}")
                    elu_inplace(sl, tmp1, tmp2)
                    if layer == 1:
                        trh = pp.tile([P, 512], F32, tag="trp", bufs=2,
                                      name=f"trh{b}")
                        nc.tensor.transpose(out=trh[:, 0:P], in_=sl,
                                            identity=identF_sb[:])
                        nc.vector.tensor_copy(out=hT_sb[:, b * P:(b + 1) * P],
                                              in_=trh[:, 0:P])

            # ---- layer 1 ----
            node_phase(xTs_sb, w1_sb, xr1_sb)
            dma_stage_to(xl1sh)
            nc.gpsimd.collective_compute(
                "AllGather", mybir.AluOpType.bypass,
                replica_groups=[list(range(8))],
                ins=[xl1sh[:]], outs=[xl1full[:]])
            edge_pass(1, xl1full, xr1_sb, k1)
            epilogue(1, recip1_sb, bb1_sb)
            # ---- layer 2 ----
            node_phase(hT_sb, w2_sb, xr2_sb)
            dma_stage_to(xl2sh)
            nc.gpsimd.collective_compute(
                "AllGather", mybir.AluOpType.bypass,
                replica_groups=[list(range(8))],
                ins=[xl2sh[:]], outs=[xl2full[:]])
            edge_pass(2, xl2full, xr2_sb, k2)
            epilogue(2, recip2_sb, bb2_sb)
            oo = out[:].rearrange("(b p) f -> p b f", p=P)
            ii = stage_o[:].rearrange("p (b f) -> p b f", f=F)
            nc.sync.dma_start(out=oo, in_=ii)

    nc.compile()
    return nc


# ---------------------------------------------------------------------------
# public entry point
# ---------------------------------------------------------------------------
_CACHE = {}
LAST_RESULTS = None


def _trace_enabled():
    import os
    return os.environ.get("GAT_TRACE", "") == "1"


def _install_trace_shim():
    """antenv.axon_hooks is absent in this image; recreate it so trace=True
    can capture NTFF profiles through the axon PJRT plugin."""
    import sys, types
    if "antenv.axon_hooks" in sys.modules:
        return
    try:
        mod = types.ModuleType("antenv.axon_hooks")
        mod._hook = None
        mod.set_axon_ntff_profile_hook = lambda h: setattr(mod, "_hook", h)
        mod.get_axon_ntff_profile_hook = lambda: mod._hook
        sys.modules["antenv.axon_hooks"] = mod
        import antenv
        antenv.axon_hooks = mod
        from trn_agent_boot.trn_boot import _ntff_profile_via_ctypes
        mod._hook = _ntff_profile_via_ctypes("/opt/axon/libaxon_pjrt.so")
        import concourse.bass_utils as bu
        bu.upload_artifacts = lambda tmpdir: str(tmpdir)
    except Exception:
        pass


def kernel(x, edge_index, Wl1, Wr1, att1, b1, Wl2, Wr2, att2, b2):
    global LAST_RESULTS
    from concourse.bass_utils import run_bass_kernel_spmd

    trace = _trace_enabled()
    if trace:
        _install_trace_shim()

    x = np.asarray(x, np.float32)
    edge_index = np.asarray(edge_index)
    N, E = x.shape[0], edge_index.shape[1]
    cfg = Cfg(N, E, nblk=49)

    per_core, struct = host_prep(cfg, x, edge_index)
    consts, meta = host_consts(cfg, Wl1, Wr1, att1, b1, Wl2, Wr2, att2, b2)

    key = (N, E, x.shape[1], struct["S_A"], struct["S_B"],
           meta["k1"], meta["k2"])
    if key not in _CACHE:
        _CACHE[key] = build_program(cfg, struct, meta["k1"], meta["k2"])
    nc = _CACHE[key]

    in_maps = []
    for k in range(8):
        m = dict(per_core[k])
        m.update(consts)
        in_maps.append(m)
    res = run_bass_kernel_spmd(nc, in_maps, core_ids=list(range(8)), trace=trace)
    LAST_RESULTS = res
    outs = [np.asarray(res.results[k]["out"]) for k in range(8)]
    full = np.concatenate(outs, axis=0)[:N].astype(np.float32)
    unperm = np.empty_like(full)
    unperm[:, meta["perm2"]] = full
    return unperm


# revision 19
# speedup vs baseline: 1.4061x; 1.1706x over previous
"""Trainium2 Bass kernel for a 2-layer GATv2 aggregator (N=50000, E=800000).

Self-contained: kernel(**inputs) takes full inputs, shards across 8
NeuronCores internally, returns the full (50000, 128) float32 output.

v2 strategy (8-core SPMD, dst-sharded):
- Channels permuted per head (positive-att first) and tables pre-scaled by the
  SIGNED att value: t~_c = att_c*(xl_c + xr_c).  Then
  att_c*leaky(t_c) = Prelu(t~_c; 0.2) for att_c>0 and min(t~, 0.2 t~)
  = Prelu(0.2*t~; 5) for att_c<0, so alpha = plain per-head sum of the
  ACT output -- one strided reduce, no per-edge att multiply.
- Messages aggregate Sum p*x~l (scaled); epilogue divides by att_c per
  channel (recip const tile).  Layer-2 weight rows pre-permuted; final
  output unpermuted on host.
- All edge-pass tiles bf16 (tables, one-hots, rhs);  eps term
  1e-16*exp(sum alpha) (replicates the oracle's segment_max-is-sum bug)
  accumulated via exact hi/lo bf16 split columns in the scatter matmul.
- dma_gather with prepare_only+trigger_dma so SWDGE desc-gen overlaps
  the DMA drain;  gather calls of 2048 edges (bf16 rows, 256 B).
"""
import numpy as np
import ml_dtypes

import concourse.bass as bass
import concourse.bacc as bacc
import concourse.mybir as mybir
from concourse.tile import TileContext

BF16 = ml_dtypes.bfloat16
F32 = mybir.dt.float32
BF = mybir.dt.bfloat16
I16 = mybir.dt.int16
PAD_DST = 200.0
P = 128
CPC = 8           # chunks per compute group
GPC = 8           # chunks per gather call (1024 idx = SWDGE ring max)
NI = GPC * 128    # indices per gather call
import os
USE_PREP = os.environ.get("GAT_PREP", "0") == "1"


class Cfg:
    def __init__(self, N, E, nblk, feat=128, heads1=2):
        self.N, self.E = N, E
        self.NBLK = nblk
        self.SHARD = nblk * P
        self.NPAD = 8 * self.SHARD
        assert self.NPAD >= N and self.NPAD % 256 == 0
        self.HALF = self.NPAD // 2
        assert self.HALF <= 32767
        self.F = feat
        self.H1 = heads1
        self.C1 = feat // heads1


def host_prep(cfg, x, edge_index):
    """Returns (per_core_inputs: list of dict, struct: dict)."""
    N, E = cfg.N, cfg.E
    src = np.concatenate([np.asarray(edge_index[0]), np.arange(N)]).astype(np.int64)
    dst = np.concatenate([np.asarray(edge_index[1]), np.arange(N)]).astype(np.int64)
    ET = src.shape[0]

    core = dst // cfg.SHARD
    block = (dst % cfg.SHARD) // P
    dloc = dst % P
    half = (src >= cfg.HALF).astype(np.int64)
    gval = (src - half * cfg.HALF).astype(np.int64)

    # group = (core, half, block); rank within group
    key = (core * 2 + half) * cfg.NBLK + block
    order = np.argsort(key, kind="stable")
    key_s = key[order]
    ngroups = 8 * 2 * cfg.NBLK
    cnt = np.bincount(key_s, minlength=ngroups)
    starts = np.zeros(ngroups + 1, np.int64)
    np.cumsum(cnt, out=starts[1:])
    rank = np.arange(ET) - starts[key_s]

    cnt3 = cnt.reshape(8, 2, cfg.NBLK)
    S_A = int(np.ceil(cnt3[:, 0, :].max() / P))
    S_B = int(np.ceil(cnt3[:, 1, :].max() / P))
    S_A, S_B = max(S_A, 1), max(S_B, 1)
    CHA = -(-(cfg.NBLK * S_A) // GPC) * GPC
    CHB = -(-(cfg.NBLK * S_B) // GPC) * GPC
    CH = CHA + CHB
    CALLS = CH // GPC

    chunk_half = np.zeros(CH, np.int64)
    chunk_block = np.zeros(CH, np.int64)
    for c in range(CH):
        if c < CHA:
            chunk_half[c] = 0
            chunk_block[c] = min(c // S_A, cfg.NBLK - 1)
        else:
            chunk_half[c] = 1
            chunk_block[c] = min((c - CHA) // S_B, cfg.NBLK - 1)

    gidx = np.zeros((8, CH, P), np.int16)
    dstl = np.full((8, CH, P), PAD_DST, np.float32)
    g_half = half[order]
    g_core = core[order]
    g_block = block[order]
    slot_base = np.where(g_half == 0, g_block * S_A, CHA + g_block * S_B)
    slot = slot_base + rank // P
    pos = rank % P
    gidx[g_core, slot, pos] = gval[order].astype(np.int16)
    dstl[g_core, slot, pos] = dloc[order].astype(np.float32)

    # wrap gather indices per call of NI: [NI//16,16].T -> [16, NI//16]
    gw = gidx.reshape(8, CALLS, NI // 16, 16).transpose(0, 1, 3, 2)
    gw = gw.transpose(0, 2, 1, 3).reshape(8, 16, CALLS * (NI // 16))
    gw = np.tile(gw, (1, 8, 1))  # replicate to 128 partitions

    struct = dict(S_A=S_A, S_B=S_B, CHA=CHA, CHB=CHB, CH=CH, CALLS=CALLS,
                  chunk_half=chunk_half, chunk_block=chunk_block)

    x_pad = np.zeros((cfg.NPAD, cfg.F), np.float32)
    x_pad[:N] = np.asarray(x, np.float32)

    per_core = []
    for k in range(8):
        per_core.append(dict(
            xTs=np.ascontiguousarray(
                x_pad[k * cfg.SHARD:(k + 1) * cfg.SHARD].T.astype(BF16)),
            gidx=np.ascontiguousarray(gw[k]),
            dstl=np.ascontiguousarray(dstl[k].T.astype(BF16)),  # [128, CH]
        ))
    return per_core, struct


def _perm_layer(Wl, Wr, att):
    """Channel perm (positive att first per head) + signed-scale weights."""
    att = np.asarray(att, np.float32)
    H, C = att.shape
    perm = np.zeros((H, C), np.int64)
    k = np.zeros(H, np.int64)
    for h in range(H):
        pos = np.where(att[h] > 0)[0]
        neg = np.where(att[h] <= 0)[0]
        perm[h] = np.concatenate([pos, neg])
        k[h] = len(pos)
    att_p = np.take_along_axis(att, perm, axis=1)
    s = att_p.reshape(-1)                    # signed scale per (permuted) chan
    flat_perm = (perm + np.arange(H)[:, None] * C).reshape(-1)
    Wl_t = np.asarray(Wl, np.float32)[:, flat_perm] * s[None, :]
    Wr_t = np.asarray(Wr, np.float32)[:, flat_perm] * s[None, :]
    return Wl_t, Wr_t, s, k, flat_perm


def host_consts(cfg, Wl1, Wr1, att1, b1, Wl2, Wr2, att2, b2):
    f = cfg.F
    Wl1t, Wr1t, s1, k1, perm1 = _perm_layer(Wl1, Wr1, att1)
    # layer2 rows permuted by perm1 (its input h is in permuted-1 order)
    Wl2t, Wr2t, s2, k2, perm2 = _perm_layer(
        np.asarray(Wl2, np.float32)[perm1], np.asarray(Wr2, np.float32)[perm1],
        att2)
    c = {}
    c["w1"] = np.hstack([Wl1t, Wr1t]).astype(BF16)
    c["w2"] = np.hstack([Wl2t, Wr2t]).astype(BF16)
    c["recip1"] = np.tile((1.0 / s1).reshape(1, f), (P, 1)).astype(np.float32)
    c["recip2"] = np.tile((1.0 / s2).reshape(1, f), (P, 1)).astype(np.float32)
    c["bb1"] = np.tile(np.asarray(b1, np.float32)[perm1].reshape(1, f), (P, 1))
    c["bb2"] = np.tile(np.asarray(b2, np.float32)[perm2].reshape(1, f), (P, 1))
    c["iotaF"] = np.tile(np.arange(P, dtype=np.float32).reshape(1, P),
                         (P, 1)).astype(BF16)
    c["iotaPB"] = np.tile(np.arange(P, dtype=np.float32).reshape(P, 1), (1, P)
                          ).astype(BF16)
    c["identB"] = np.eye(P, dtype=np.float32).astype(BF16)
    c["identF"] = np.eye(P, dtype=np.float32)
    meta = dict(k1=tuple(int(v) for v in k1), k2=tuple(int(v) for v in k2),
                perm2=perm2)
    return c, meta


def _ap(base, layout):
    return bass.AP(base.tensor, base.offset, [list(d) for d in layout])


def build_program(cfg, struct, k1, k2):
    NBLK, SHARD, NPAD, HALF, F = cfg.NBLK, cfg.SHARD, cfg.NPAD, cfg.HALF, cfg.F
    CH, CALLS = struct["CH"], struct["CALLS"]
    chunk_half, chunk_block = struct["chunk_half"], struct["chunk_block"]
    RW = F + 8

    nc = bacc.Bacc("TRN2", target_bir_lowering=False, debug=False,
                   num_devices=8, num_swdge_queues=4)

    xTs = nc.dram_tensor("xTs", [P, SHARD], BF, kind="ExternalInput")
    gidx = nc.dram_tensor("gidx", [P, CALLS * (NI // 16)], I16, kind="ExternalInput")
    dstl = nc.dram_tensor("dstl", [P, CH], BF, kind="ExternalInput")
    w1 = nc.dram_tensor("w1", [P, 2 * F], BF, kind="ExternalInput")
    w2 = nc.dram_tensor("w2", [P, 2 * F], BF, kind="ExternalInput")
    recip1 = nc.dram_tensor("recip1", [P, F], F32, kind="ExternalInput")
    recip2 = nc.dram_tensor("recip2", [P, F], F32, kind="ExternalInput")
    bb1 = nc.dram_tensor("bb1", [P, F], F32, kind="ExternalInput")
    bb2 = nc.dram_tensor("bb2", [P, F], F32, kind="ExternalInput")
    iotaF = nc.dram_tensor("iotaF", [P, P], BF, kind="ExternalInput")
    iotaPB = nc.dram_tensor("iotaPB", [P, P], BF, kind="ExternalInput")
    identB = nc.dram_tensor("identB", [P, P], BF, kind="ExternalInput")
    identF = nc.dram_tensor("identF", [P, P], F32, kind="ExternalInput")
    out = nc.dram_tensor("out", [SHARD, F], F32, kind="ExternalOutput")

    eq = mybir.AluOpType.is_equal
    mul = mybir.AluOpType.mult
    AF = mybir.ActivationFunctionType
    AX = mybir.AxisListType.X

    with TileContext(nc) as tc:
        with (
            tc.tile_pool(name="const", bufs=1) as cpool,
            tc.tile_pool(name="big", bufs=1) as bigp,
            tc.tile_pool(name="work", bufs=1) as wp,
            tc.tile_pool(name="psum", bufs=1, space="PSUM") as pp,
            tc.tile_pool(name="dram", bufs=1, space="DRAM") as dp,
        ):
            def load_const(t, shape, dt):
                s = cpool.tile(shape, dt, name=t.name + "_sb")
                nc.sync.dma_start(out=s[:], in_=t[:])
                return s
            w1_sb = load_const(w1, [P, 2 * F], BF)
            w2_sb = load_const(w2, [P, 2 * F], BF)
            recip1_sb = load_const(recip1, [P, F], F32)
            recip2_sb = load_const(recip2, [P, F], F32)
            bb1_sb = load_const(bb1, [P, F], F32)
            bb2_sb = load_const(bb2, [P, F], F32)
            iotaF_sb = load_const(iotaF, [P, P], BF)
            iotaPB_sb = load_const(iotaPB, [P, P], BF)
            identB_sb = load_const(identB, [P, P], BF)
            identF_sb = load_const(identF, [P, P], F32)
            xTs_sb = bigp.tile([P, SHARD], BF, name="xTs_sb")
            nc.sync.dma_start(out=xTs_sb[:], in_=xTs[:])
            gidx_sb = bigp.tile([P, CALLS * (NI // 16)], I16, name="gidx_sb")
            nc.sync.dma_start(out=gidx_sb[:], in_=gidx[:])
            dstl_sb = bigp.tile([P, CH], BF, name="dstl_sb")
            nc.sync.dma_start(out=dstl_sb[:], in_=dstl[:])

            lneps_sb = cpool.tile([P, 1], F32, name="lneps_sb")
            nc.vector.memset(lneps_sb[:], float(np.log(1e-16)))
            xr1_sb = bigp.tile([P, SHARD], BF, name="xr1_sb")
            xr2_sb = bigp.tile([P, SHARD], BF, name="xr2_sb")
            hT_sb = bigp.tile([P, SHARD], BF, name="hT_sb")
            hacc = bigp.tile([P, NBLK * RW], F32, name="hacc")
            stage = bigp.tile([P, SHARD], BF, name="stage")      # xl (bf16)
            stage_o = bigp.tile([P, SHARD], F32, name="stage_o")  # epilogue f32

            xl1sh = dp.tile([SHARD, F], BF, name="xl1sh")
            xl1full = dp.tile([NPAD, F], BF, name="xl1full", addr_space="Shared")
            xl2sh = dp.tile([SHARD, F], BF, name="xl2sh")
            xl2full = dp.tile([NPAD, F], BF, name="xl2full", addr_space="Shared")

            dma_sem = nc.alloc_semaphore("gat_dma")

            def node_phase(src_sb, w_sb, xr_dst):
                for j in range(NBLK):
                    mm = pp.tile([P, 8, P], F32, tag="txr", bufs=2, name=f"mm{j}")
                    mf = mm[:].rearrange("p c f -> p (c f)")
                    nc.tensor.matmul(out=mf[:, 0:2 * F],
                                     lhsT=src_sb[:, j * P:(j + 1) * P],
                                     rhs=w_sb[:], start=True, stop=True)
                    nc.vector.tensor_copy(out=stage[:, j * F:(j + 1) * F],
                                          in_=mf[:, 0:F])
                    nc.vector.tensor_copy(out=xr_dst[:, j * P:(j + 1) * P],
                                          in_=mf[:, F:2 * F])

            def dma_stage_to(dram_tile):
                o = dram_tile[:].rearrange("(b p) f -> p b f", p=P)
                i = stage[:].rearrange("p (b f) -> p b f", f=F)
                nc.sync.dma_start(out=o, in_=i)

            def edge_pass(layer, table, xr_sb, ks):
                HN = cfg.H1 if layer == 1 else 1
                CW = F // HN
                RWB = F + 3 * HN
                nc.vector.memset(hacc[:], 0.0)
                bp = None
                for g in range(CALLS):
                    cb0 = g * GPC
                    hf = int(chunk_half[cb0])
                    tab = table[:][0:HALF, :] if hf == 0 else table[:][HALF:NPAD, :]
                    xg = wp.tile([P, GPC, F], BF, tag="xg", bufs=3,
                                 name=f"xg{layer}_{g}")
                    if USE_PREP:
                        nc.gpsimd.dma_gather(
                            out_ap=xg[:], in_ap=tab,
                            idxs_ap=gidx_sb[:, g * (NI // 16):(g + 1) * (NI // 16)],
                            num_idxs=NI, num_idxs_reg=NI, elem_size=F,
                            prepare_only=True, sem=dma_sem, queue_num=0)
                        nc.gpsimd.trigger_dma(count=None)
                    else:
                        nc.gpsimd.dma_gather(
                            out_ap=xg[:], in_ap=tab,
                            idxs_ap=gidx_sb[:, g * (NI // 16):(g + 1) * (NI // 16)],
                            num_idxs=NI, num_idxs_reg=NI, elem_size=F,
                            queue_num=g % 4)
                    for sub in range(GPC // CPC):
                        cb = cb0 + sub * CPC
                        d8 = dstl_sb[:, cb:cb + CPC]
                        # one-hot QT[e, n] (lhsT for scatter)
                        qt = wp.tile([P, CPC, P], BF, tag="qt", bufs=3,
                                     name=f"qt{layer}_{cb}")
                        nc.vector.tensor_tensor(
                            out=qt[:], in0=d8.to_broadcast([P, CPC, P]),
                            in1=_ap(iotaF_sb[:],
                                    [iotaF_sb[:].ap[0], [0, CPC], [1, P]]),
                            op=eq)
                        # Q[n, e] one-hot = PE transpose of QT; copy to SBUF on ACT
                        trp = pp.tile([P, CPC * P], BF, tag="trp", bufs=2,
                                      name=f"trp{layer}_{cb}")
                        for c in range(CPC):
                            nc.tensor.transpose(
                                out=trp[:, c * P:(c + 1) * P],
                                in_=qt[:, c, :], identity=identB_sb[:])
                        q = wp.tile([P, CPC, P], BF, tag="q", bufs=3,
                                    name=f"q{layer}_{cb}")
                        nc.scalar.activation(
                            out=q[:], in_=trp[:].rearrange("p (c f) -> p c f", f=P),
                            func=AF.Copy)
                        # t~ = Q.T @ xr (+ xg via identity matmul), PSUM f32
                        txr = pp.tile([P, CPC, P], F32, tag="txr", bufs=2,
                                      name=f"txr{layer}_{cb}")
                        for c in range(CPC):
                            blk = int(chunk_block[cb + c])
                            nc.tensor.matmul(
                                out=txr[:, c, :], lhsT=q[:, c, :],
                                rhs=xr_sb[:, blk * P:(blk + 1) * P],
                                start=True, stop=False)
                            nc.tensor.matmul(
                                out=txr[:, c, :], lhsT=identB_sb[:],
                                rhs=xg[:, sub * CPC + c, :],
                                start=False, stop=True)
                        # att_c*leaky(t_c): Prelu(x;.2) pos block,
                        # Prelu(.2x;5)=min(x,.2x) neg block, per head
                        lr = wp.tile([P, CPC, F], BF, tag="lr", bufs=2,
                                     name=f"lr{layer}_{cb}")
                        tb = txr[:]
                        lb = lr[:]
                        for h in range(HN):
                            kh = ks[h]
                            if kh > 0:
                                ap_i = bass.AP(tb.tensor, tb.offset + h * CW,
                                               [list(tb.ap[0]), [F, CPC], [1, kh]])
                                ap_o = bass.AP(lb.tensor, lb.offset + h * CW,
                                               [list(lb.ap[0]), [F, CPC], [1, kh]])
                                nc.scalar.activation(out=ap_o, in_=ap_i,
                                                     func=AF.Prelu, alpha=0.2)
                            if kh < CW:
                                off = h * CW + kh
                                ap_i = bass.AP(tb.tensor, tb.offset + off,
                                               [list(tb.ap[0]), [F, CPC],
                                                [1, CW - kh]])
                                ap_o = bass.AP(lb.tensor, lb.offset + off,
                                               [list(lb.ap[0]), [F, CPC],
                                                [1, CW - kh]])
                                nc.scalar.activation(out=ap_o, in_=ap_i,
                                                     func=AF.Prelu, alpha=5.0,
                                                     scale=0.2)
                        # alpha = per-head sum
                        al = wp.tile([P, CPC * HN], F32, tag="al", bufs=2,
                                     name=f"al{layer}_{cb}")
                        nc.vector.reduce_sum(
                            out=al[:],
                            in_=lr[:].rearrange("p c (h s) -> p c h s", s=CW),
                            axis=AX)
                        # p = exp(alpha) (bf16)
                        pe = wp.tile([P, CPC * HN], BF, tag="pe", bufs=2,
                                     name=f"pe{layer}_{cb}")
                        nc.scalar.activation(out=pe[:], in_=al[:], func=AF.Exp)
                        # thi = bf16(al + 8): snaps al to the exact bf16 grid
                        thi = wp.tile([P, CPC * HN], BF, tag="thi", bufs=2,
                                      name=f"thi{layer}_{cb}")
                        nc.vector.tensor_scalar_add(out=thi[:], in0=al[:],
                                                    scalar1=8.0)
                        # rhs = [p*xg | p | hi | lo]
                        rhs = wp.tile([P, CPC, RWB], BF, tag="rhs", bufs=3,
                                      name=f"rhs{layer}_{cb}")
                        rb = rhs[:]
                        xb = xg[:, sub * CPC:(sub + 1) * CPC, :]
                        pb = pe[:]
                        nc.vector.tensor_tensor(
                            out=_ap(rb, [rb.ap[0], [RWB, CPC], [CW, HN], [1, CW]]),
                            in0=_ap(xb, [xb.ap[0], [F, CPC], [CW, HN], [1, CW]]),
                            in1=_ap(pb, [pb.ap[0], [HN, CPC], [1, HN], [0, CW]]),
                            op=mul)
                        pc_out = bass.AP(rb.tensor, rb.offset + F,
                                         [list(rb.ap[0]), [RWB, CPC], [1, HN]])
                        nc.vector.tensor_copy(
                            out=pc_out, in_=pb.rearrange("p (c h) -> p c h", h=HN))
                        hi_out = bass.AP(rb.tensor, rb.offset + F + HN,
                                         [list(rb.ap[0]), [RWB, CPC], [1, HN]])
                        nc.vector.tensor_scalar_add(
                            out=hi_out, in0=thi[:].rearrange(
                                "p (c h) -> p c h", h=HN), scalar1=-8.0)
                        lo_out = bass.AP(rb.tensor, rb.offset + F + 2 * HN,
                                         [list(rb.ap[0]), [RWB, CPC], [1, HN]])
                        nc.vector.tensor_tensor(
                            out=lo_out,
                            in0=al[:].rearrange("p (c h) -> p c h", h=HN),
                            in1=hi_out, op=mybir.AluOpType.subtract)
                        # scatter matmuls, PSUM-accumulated per block segment
                        for c in range(CPC):
                            ci = cb + c
                            blk = int(chunk_block[ci])
                            seg_start = ci == 0 or chunk_block[ci - 1] != blk
                            seg_end = ci == CH - 1 or chunk_block[ci + 1] != blk
                            if seg_start:
                                bp = pp.tile([P, RWB], F32, tag="bp", bufs=2,
                                             name=f"bp{layer}_{ci}")
                            nc.tensor.matmul(
                                out=bp[:], lhsT=qt[:, c, :], rhs=rhs[:, c, :],
                                start=seg_start, stop=seg_end)
                            if seg_end:
                                nc.vector.tensor_add(
                                    out=hacc[:, blk * RW:blk * RW + RWB],
                                    in0=hacc[:, blk * RW:blk * RW + RWB],
                                    in1=bp[:])

            def elu_inplace(sl, tmp1, tmp2):
                nc.vector.tensor_scalar_min(out=tmp1[:], in0=sl, scalar1=0.0)
                nc.scalar.activation(out=tmp2[:], in_=tmp1[:], func=AF.Exp)
                nc.vector.tensor_scalar_max(out=sl, in0=sl, scalar1=0.0)
                nc.vector.tensor_add(out=sl, in0=sl, in1=tmp2[:])
                nc.vector.tensor_scalar_add(out=sl, in0=sl, scalar1=-1.0)

            def epilogue(layer, recip_sb, bb_sb):
                HN = cfg.H1 if layer == 1 else 1
                CW = F // HN
                RWB = F + 3 * HN
                for b in range(NBLK):
                    hb = hacc[:, b * RW:b * RW + RWB]
                    sa = wp.tile([P, HN], F32, tag="sa", bufs=2,
                                 name=f"sa{layer}_{b}")
                    nc.vector.tensor_add(
                        out=sa[:], in0=hacc[:, b * RW + F + HN:b * RW + F + 2 * HN],
                        in1=hacc[:, b * RW + F + 2 * HN:b * RW + F + 3 * HN])
                    eps = wp.tile([P, HN], F32, tag="eps", bufs=2,
                                  name=f"eps{layer}_{b}")
                    nc.scalar.activation(out=eps[:], in_=sa[:], func=AF.Exp,
                                         bias=lneps_sb[:, 0:1])
                    den = wp.tile([P, HN], F32, tag="den", bufs=2,
                                  name=f"den{layer}_{b}")
                    nc.vector.tensor_add(
                        out=den[:], in0=hacc[:, b * RW + F:b * RW + F + HN],
                        in1=eps[:])
                    rec = wp.tile([P, HN], F32, tag="rec", bufs=2,
                                  name=f"rec{layer}_{b}")
                    nc.vector.reciprocal(out=rec[:], in_=den[:])
                    # scale[n, c] = rec[n, head(c)] * recip_att[c]
                    sc = wp.tile([P, F], F32, tag="sc", bufs=2,
                                 name=f"sc{layer}_{b}")
                    rb = rec[:]
                    nc.vector.tensor_tensor(
                        out=sc[:].rearrange("p (h s) -> p h s", s=CW),
                        in0=_ap(rb, [rb.ap[0], [1, HN], [0, CW]]),
                        in1=recip_sb[:].rearrange("p (h s) -> p h s", s=CW),
                        op=mul)
                    sl = stage_o[:, b * F:(b + 1) * F]
                    nc.vector.tensor_tensor(out=sl, in0=hacc[:, b * RW:b * RW + F],
                                            in1=sc[:], op=mul)
                    nc.vector.tensor_add(out=sl, in0=sl, in1=bb_sb[:])
                    tmp1 = wp.tile([P, F], F32, tag="tmp1", bufs=2,
                                   name=f"t1_{layer}_{b}")
                    tmp2 = wp.tile([P, F], F32, tag="tmp2", bufs=2,
                                   name=f"t2_{layer}_{b}")
                    elu_inplace(sl, tmp1, tmp2)
                    if layer == 1:
                        trh = pp.tile([P, 512], F32, tag="trp", bufs=2,
                                      name=f"trh{b}")
                        nc.tensor.transpose(out=trh[:, 0:P], in_=sl,
                                            identity=identF_sb[:])
                        nc.vector.tensor_copy(out=hT_sb[:, b * P:(b + 1) * P],
                                              in_=trh[:, 0:P])

            # ---- layer 1 ----
            node_phase(xTs_sb, w1_sb, xr1_sb)
            dma_stage_to(xl1sh)
            nc.gpsimd.collective_compute(
                "AllGather", mybir.AluOpType.bypass,
                replica_groups=[list(range(8))],
                ins=[xl1sh[:]], outs=[xl1full[:]])
            edge_pass(1, xl1full, xr1_sb, k1)
            epilogue(1, recip1_sb, bb1_sb)
            # ---- layer 2 ----
            node_phase(hT_sb, w2_sb, xr2_sb)
            dma_stage_to(xl2sh)
            nc.gpsimd.collective_compute(
                "AllGather", mybir.AluOpType.bypass,
                replica_groups=[list(range(8))],
                ins=[xl2sh[:]], outs=[xl2full[:]])
            edge_pass(2, xl2full, xr2_sb, k2)
            epilogue(2, recip2_sb, bb2_sb)
            oo = out[:].rearrange("(b p) f -> p b f", p=P)
            ii = stage_o[:].rearrange("p (b f) -> p b f", f=F)
            nc.sync.dma_start(out=oo, in_=ii)

    nc.compile()
    return nc


# ---------------------------------------------------------------------------
# public entry point
# ---------------------------------------------------------------------------
_CACHE = {}
LAST_RESULTS = None


def _trace_enabled():
    import os
    return os.environ.get("GAT_TRACE", "") == "1"


def _install_trace_shim():
    """antenv.axon_hooks is absent in this image; recreate it so trace=True
    can capture NTFF profiles through the axon PJRT plugin."""
    import sys, types
    if "antenv.axon_hooks" in sys.modules:
        return
    try:
        mod = types.ModuleType("antenv.axon_hooks")
        mod._hook = None
        mod.set_axon_ntff_profile_hook = lambda h: setattr(mod, "_hook", h)
        mod.get_axon_ntff_profile_hook = lambda: mod._hook
        sys.modules["antenv.axon_hooks"] = mod
        import antenv
        antenv.axon_hooks = mod
        from trn_agent_boot.trn_boot import _ntff_profile_via_ctypes
        mod._hook = _ntff_profile_via_ctypes("/opt/axon/libaxon_pjrt.so")
        import concourse.bass_utils as bu
        bu.upload_artifacts = lambda tmpdir: str(tmpdir)
    except Exception:
        pass


def kernel(x, edge_index, Wl1, Wr1, att1, b1, Wl2, Wr2, att2, b2):
    global LAST_RESULTS
    from concourse.bass_utils import run_bass_kernel_spmd

    trace = _trace_enabled()
    if trace:
        _install_trace_shim()

    x = np.asarray(x, np.float32)
    edge_index = np.asarray(edge_index)
    N, E = x.shape[0], edge_index.shape[1]
    cfg = Cfg(N, E, nblk=49)

    per_core, struct = host_prep(cfg, x, edge_index)
    consts, meta = host_consts(cfg, Wl1, Wr1, att1, b1, Wl2, Wr2, att2, b2)

    key = (N, E, x.shape[1], struct["S_A"], struct["S_B"],
           meta["k1"], meta["k2"])
    if key not in _CACHE:
        _CACHE[key] = build_program(cfg, struct, meta["k1"], meta["k2"])
    nc = _CACHE[key]

    in_maps = []
    for k in range(8):
        m = dict(per_core[k])
        m.update(consts)
        in_maps.append(m)
    res = run_bass_kernel_spmd(nc, in_maps, core_ids=list(range(8)), trace=trace)
    LAST_RESULTS = res
    outs = [np.asarray(res.results[k]["out"]) for k in range(8)]
    full = np.concatenate(outs, axis=0)[:N].astype(np.float32)
    unperm = np.empty_like(full)
    unperm[:, meta["perm2"]] = full
    return unperm
